# revision 1
# baseline (speedup 1.0000x reference)
"""v3: interleaved projections+attention, bf16 weights/x, causal shrink,
pool-engine denominator broadcast, software-pipelined PV.

Structure per core (core = 2*b + head-group g, 8 heads x 64 dims):
  upfront: load wk,x -> K(0); wq -> Q(0); wv -> V(0); wp
  for tg in 0..3:
      B(tg): attention over t-group tg, with K/Q/V projection groups of
             t-group tg+1 sprinkled between head-pairs (keeps PE fed while
             ACT chews exps; ACT of B overlaps proj matmuls).
      out-proj(tg)
Score matmuls f32r (kt/qt f32r); everything else bf16 except PSUM paths.
Causal shrink: s-tile si only reaches t >= 128*si; score/exp start at
min(toff, 256) (f32r matmul needs free>=256 for full rate), pv at toff.
Denominator: reciprocal of the ones-column row of psy, broadcast across
64 partitions on the (idle) Pool engine via partition_broadcast.
"""

import sys

if "/opt/trn_rl_repo" not in sys.path:
    sys.path.insert(0, "/opt/trn_rl_repo")

import ml_dtypes
import numpy as np

import concourse.bacc as bacc
import concourse.mybir as mybir
from concourse.tile import TileContext
from concourse.bass_utils import run_bass_kernel_spmd

B, T, C = 4, 2048, 1024
H_LOC = 8
D = 64
DL = H_LOC * D
P = 128
NF = 512
N_TG = T // NF
N_CS = C // P
SCALE = 1.0 / 8.0

F32 = mybir.dt.float32
F32R = mybir.dt.float32r
BF16 = mybir.dt.bfloat16
EXP = mybir.ActivationFunctionType.Exp


def build_nc(pp_bufs=3, pss_bufs=3, ex_bufs=6, psy_bufs=2,
             alloc_mode="stack", repeat=1, interleave=True, sprinkle=3,
             fuse_exp=False, mask_pool=False, bcast_pool=True):
    nc = bacc.Bacc("TRN2", target_bir_lowering=False, debug=False, num_devices=8)

    xT = nc.dram_tensor("xT", [C, T], BF16, kind="ExternalInput")
    wq = nc.dram_tensor("wq", [C, DL], BF16, kind="ExternalInput")
    wk = nc.dram_tensor("wk", [C, DL], BF16, kind="ExternalInput")
    wv = nc.dram_tensor("wv", [C, DL], BF16, kind="ExternalInput")
    wp = nc.dram_tensor("wp", [DL, C], BF16, kind="ExternalInput")
    bq = nc.dram_tensor("bq", [P, DL // P], F32, kind="ExternalInput")
    bk = nc.dram_tensor("bk", [P, DL // P], F32, kind="ExternalInput")
    bv = nc.dram_tensor("bv", [P, DL], BF16, kind="ExternalInput")
    ones_in = nc.dram_tensor("ones", [1, D], F32R, kind="ExternalInput")
    outT = nc.dram_tensor("outT", [C, T], F32, kind="ExternalOutput")

    with TileContext(nc, pool_alloc_mode=alloc_mode) as tc:
        with (
            tc.tile_pool(name="persist", bufs=1) as persist,
            tc.tile_pool(name="wpool", bufs=1) as wpool,
            tc.tile_pool(name="attp", bufs=4) as attp,
            tc.tile_pool(name="ocpp", bufs=2) as ocpp,
            tc.tile_pool(name="att1", bufs=1) as att1,
            tc.tile_pool(name="att2", bufs=2) as att2,
            tc.tile_pool(name="xpool", bufs=1) as xpool,
            tc.tile_pool(name="psum", bufs=2, space="PSUM") as psum,
        ):
            def emit(rep):
                kt_g = [persist.tile([P, DL // P, NF], F32R, tag=f"kt{g}",
                                     name=f"kt{g}_{rep}")
                        for g in range(N_TG)]
                qt_g = [persist.tile([P, DL // P, NF], F32R, tag=f"qt{g}",
                                     name=f"qt{g}_{rep}")
                        for g in range(N_TG)]
                va_g = [persist.tile([P, 4, H_LOC, D + 1], BF16, tag=f"va{g}",
                                     name=f"va{g}_{rep}")
                        for g in range(N_TG)]
                bq_c = persist.tile([P, DL // P], F32, tag="bq")
                bk_c = persist.tile([P, DL // P], F32, tag="bk")
                bv_b = persist.tile([P, DL], BF16, tag="bv")
                ones = persist.tile([P, D], F32R, tag="ones")
                dmask = persist.tile([P, 2 * P], BF16, tag="dmask")

                nc.sync.dma_start(out=bq_c[:], in_=bq[:])
                nc.sync.dma_start(out=bk_c[:], in_=bk[:])
                nc.sync.dma_start(out=bv_b[:], in_=bv[:])
                nc.sync.dma_start(out=ones[D : D + 1, :], in_=ones_in[:])
                # dmask[p, j] = 1 if j - 128 >= p else 0  (diag window at 128)
                nc.vector.memset(dmask[:], 1.0)
                nc.gpsimd.affine_select(
                    out=dmask[:],
                    in_=dmask[:],
                    compare_op=mybir.AluOpType.is_ge,
                    fill=0.0,
                    base=-P,
                    channel_multiplier=-1,
                    pattern=[[1, 2 * P]],
                )
                for g in range(N_TG):
                    nc.vector.memset(va_g[g][:, :, :, D : D + 1], 1.0)
                if bcast_pool and rep == 0:
                    # partition_broadcast lives in the gpsimd `attn` library;
                    # load it after the (native) affine_select mask init
                    from concourse import library_config

                    nc.gpsimd.load_library(library_config.attn)

                # -------- weights + x --------
                wk_sb = wpool.tile([P, N_CS, DL], BF16, tag="wk")
                nc.sync.dma_start(
                    out=wk_sb[:], in_=wk.ap().rearrange("(s p) d -> p s d", p=P)
                )
                xt_c = [xpool.tile([P, T], BF16, tag=f"x{cs}", name=f"x{cs}_{rep}")
                        for cs in range(N_CS)]
                for cs in range(N_CS):
                    nc.sync.dma_start(
                        out=xt_c[cs][:], in_=xT.ap()[cs * P : (cs + 1) * P, :]
                    )
                wq_sb = wpool.tile([P, N_CS, DL], BF16, tag="wq")
                nc.sync.dma_start(
                    out=wq_sb[:], in_=wq.ap().rearrange("(s p) d -> p s d", p=P)
                )
                wv_sb = wpool.tile([P, N_CS, DL], BF16, tag="wv")
                nc.sync.dma_start(
                    out=wv_sb[:], in_=wv.ap().rearrange("(s p) d -> p s d", p=P)
                )
                wp_sb = wpool.tile([P, DL // P, C], BF16, tag="wp")
                nc.sync.dma_start(
                    out=wp_sb[:], in_=wp.ap().rearrange("(s p) c -> p s c", p=P)
                )

                def k_group(g, dt_i):
                    ps = psum.tile([P, NF], F32, tag="pp", bufs=pp_bufs)
                    for cs in range(N_CS):
                        nc.tensor.matmul(
                            ps[:],
                            wk_sb[:, cs, dt_i * P : (dt_i + 1) * P],
                            xt_c[cs][:, g * NF : (g + 1) * NF],
                            start=(cs == 0),
                            stop=(cs == N_CS - 1),
                        )
                    nc.vector.tensor_scalar_add(
                        kt_g[g][:, dt_i, :], ps[:], bk_c[:, dt_i : dt_i + 1]
                    )

                def q_group(g, dt_i):
                    ps = psum.tile([P, NF], F32, tag="pp", bufs=pp_bufs)
                    for cs in range(N_CS):
                        nc.tensor.matmul(
                            ps[:],
                            wq_sb[:, cs, dt_i * P : (dt_i + 1) * P],
                            xt_c[cs][:, g * NF : (g + 1) * NF],
                            start=(cs == 0),
                            stop=(cs == N_CS - 1),
                        )
                    nc.vector.tensor_scalar_add(
                        qt_g[g][:, dt_i, :], ps[:], bq_c[:, dt_i : dt_i + 1]
                    )

                def v_group(st):
                    ps = psum.tile([P, NF], F32, tag="pp", bufs=pp_bufs)
                    for cs in range(N_CS):
                        nc.tensor.matmul(
                            ps[:],
                            xt_c[cs][:, st * P : (st + 1) * P],
                            wv_sb[:, cs, :],
                            start=(cs == 0),
                            stop=(cs == N_CS - 1),
                        )
                    nc.vector.tensor_add(
                        va_g[st // 4][:, st % 4, :, 0:D],
                        ps[:].rearrange("p (h d) -> p h d", d=D),
                        bv_b[:].rearrange("p (h d) -> p h d", d=D),
                    )

                def a_groups(g):
                    for dt_i in range(DL // P):
                        yield lambda dt_i=dt_i: k_group(g, dt_i)
                    for dt_i in range(DL // P):
                        yield lambda dt_i=dt_i: q_group(g, dt_i)
                    for j in range(4):
                        yield lambda j=j: v_group(4 * g + j)

                ytn_g = {}

                def op_group(tg, ct):
                    ytn = ytn_g[tg]
                    pso = psum.tile([P, NF], F32, tag="pp", bufs=pp_bufs)
                    for js in range(DL // P):
                        nc.tensor.matmul(
                            pso[:],
                            wp_sb[:, js, ct * P : (ct + 1) * P],
                            ytn[:, js, :],
                            start=(js == 0),
                            stop=(js == DL // P - 1),
                        )
                    ocp = ocpp.tile([P, NF], F32, tag="ocp")
                    nc.vector.tensor_copy(ocp[:], pso[:])
                    nc.sync.dma_start(
                        out=outT.ap()[
                            ct * P : (ct + 1) * P, tg * NF : (tg + 1) * NF
                        ],
                        in_=ocp[:],
                    )

                def op_groups(tg):
                    for ct in range(C // P):
                        yield lambda ct=ct: op_group(tg, ct)

                def emit_att(tg, feeder):
                    n_s = 4 * (tg + 1)
                    qt = qt_g[tg]
                    ytn = att2.tile([P, DL // P, NF], BF16, tag=f"ytn{tg}",
                                    name=f"ytn{tg}_{rep}", bufs=1)
                    ytn_g[tg] = ytn
                    n_feed = len(feeder)
                    fed = 0
                    for hp in range(H_LOC // 2):
                        pair = (2 * hp, 2 * hp + 1)
                        psy = {
                            h: psum.tile([D + 1, NF], F32, tag="psy",
                                         name=f"psy{h}_t{tg}_{rep}",
                                         bufs=psy_bufs)
                            for h in pair
                        }

                        def flush_pv(si, exs, toff):
                            for h in pair:
                                nc.tensor.matmul(
                                    psy[h][:, toff:] if toff else psy[h],
                                    va_g[si // 4][:, si % 4, h, :],
                                    exs[h][:, toff:],
                                    start=(si == 0),
                                    stop=(si == n_s - 1),
                                )

                        mask_mul = (nc.gpsimd.tensor_mul if mask_pool
                                    else nc.vector.tensor_mul)
                        pend = None  # delayed pv args: (si, exs, toff)
                        for si in range(n_s):
                            toff = max(0, (si - 4 * tg) * P)
                            ts = min(toff, NF - 256)
                            if fuse_exp:
                                # both heads' scores in one 2-bank psum tile;
                                # single exp over [128, 2*(NF-ts)]
                                ps2 = psum.tile([P, 2, NF], F32, tag="pss",
                                                name="ps2", bufs=pss_bufs // 2)
                                pss = {h: ps2[:, i, :] for i, h in enumerate(pair)}
                            else:
                                pss = {}
                                for h in pair:
                                    pss[h] = psum.tile([P, NF], F32, tag="pss",
                                                       name="pss", bufs=pss_bufs)
                            for h in pair:
                                rlo = D * (h % 2)
                                hs = h // 2
                                nc.tensor.matmul(
                                    pss[h][:, ts:],
                                    kt_g[si // 4][
                                        rlo : rlo + D, hs,
                                        (si % 4) * P : (si % 4 + 1) * P
                                    ],
                                    qt[rlo : rlo + D, hs, ts:],
                                    start=True,
                                    stop=True,
                                )
                            exs = {}
                            if fuse_exp:
                                ex2 = attp.tile([P, 2, NF], BF16, tag="ex",
                                                bufs=ex_bufs)
                                nc.scalar.activation(
                                    ex2[:, :, ts:], ps2[:, :, ts:], EXP,
                                    scale=SCALE
                                )
                                for i, h in enumerate(pair):
                                    if si >= 4 * tg:
                                        mask_mul(
                                            ex2[:, i, ts : toff + P],
                                            ex2[:, i, ts : toff + P],
                                            dmask[:, P + ts - toff : 2 * P],
                                        )
                                    exs[h] = ex2[:, i, :]
                            else:
                                for h in pair:
                                    ex = attp.tile([P, NF], BF16, tag="ex",
                                                   bufs=ex_bufs)
                                    nc.scalar.activation(
                                        ex[:, ts:], pss[h][:, ts:], EXP,
                                        scale=SCALE
                                    )
                                    if si >= 4 * tg:  # diagonal: zero s > t
                                        mask_mul(
                                            ex[:, ts : toff + P],
                                            ex[:, ts : toff + P],
                                            dmask[:, P + ts - toff : 2 * P],
                                        )
                                    exs[h] = ex
                            if pend is not None:
                                flush_pv(*pend)
                            pend = (si, exs, toff)
                        flush_pv(*pend)

                        def _norm(h):
                            hs = h // 2
                            rec = att1.tile([D, NF], F32, tag="rec")
                            if bcast_pool:
                                rec1 = att1.tile([1, NF], F32, tag="dt")
                                nc.vector.reciprocal(
                                    rec1[:], psy[h][D : D + 1, :]
                                )
                                nc.gpsimd.partition_broadcast(
                                    rec[:], rec1[0:1, :], channels=D
                                )
                            else:
                                den = att1.tile([D + 1, NF], F32R, tag="dt")
                                nc.vector.tensor_copy(
                                    den[D : D + 1, :], psy[h][D : D + 1, :]
                                )
                                pbc = psum.tile(
                                    [D, NF], F32, name="pbc",
                                    tag="pp" if fuse_exp else "pss",
                                    bufs=pp_bufs if fuse_exp else pss_bufs,
                                )
                                nc.tensor.matmul(
                                    pbc[:],
                                    ones[D : D + 1, :],
                                    den[D : D + 1, :],
                                    start=True,
                                    stop=True,
                                )
                                nc.vector.reciprocal(rec[:], pbc[:])
                            if h % 2 == 0:
                                nc.vector.tensor_mul(
                                    ytn[0:D, hs, :], psy[h][0:D, :], rec[:]
                                )
                            else:
                                tmp = att1.tile([D, NF], BF16, tag="tm")
                                nc.vector.tensor_mul(
                                    tmp[:], psy[h][0:D, :], rec[:]
                                )
                                nc.sync.dma_start(
                                    out=ytn[D:P, hs, :], in_=tmp[:]
                                )

                        for h in pair:
                            _norm(h)
                        # drain an even share of the feeder after each head-pair
                        want = (n_feed * (hp + 1) + 3) // 4
                        while fed < want:
                            feeder[fed]()
                            fed += 1

                # -------- schedule --------
                for dt_i in range(DL // P):
                    k_group(0, dt_i)
                for dt_i in range(DL // P):
                    q_group(0, dt_i)
                for j in range(4):
                    v_group(j)
                for tg in range(N_TG):
                    if tg + 1 < N_TG:
                        feeder = list(a_groups(tg + 1))
                    else:
                        # B3 has no next projections; feed it the deferred
                        # out-projections of t-groups 0..2
                        feeder = [
                            th for t in range(N_TG - 1) for th in op_groups(t)
                        ]
                    if not interleave:
                        for th in feeder:
                            th()
                        feeder = []
                    emit_att(tg, feeder)
                for th in op_groups(N_TG - 1):
                    th()

            emit(0)
            for rep in range(1, repeat):
                emit(rep)

    nc.compile()
    return nc


def _prep_inputs(x, Wq, bq, Wk, bk, Wv, bv, Wp):
    bf = ml_dtypes.bfloat16
    in_maps = []
    for b in range(B):
        xt = np.ascontiguousarray(x[b].T).astype(bf)
        for g in range(2):
            sl = slice(g * DL, (g + 1) * DL)
            in_maps.append(
                {
                    "xT": xt,
                    "wq": np.ascontiguousarray(Wq[:, sl]).astype(bf),
                    "wk": np.ascontiguousarray(Wk[:, sl]).astype(bf),
                    "wv": np.ascontiguousarray(Wv[:, sl]).astype(bf),
                    "wp": np.ascontiguousarray(Wp[sl, :]).astype(bf),
                    "bq": np.ascontiguousarray(bq[sl].reshape(DL // P, P).T),
                    "bk": np.ascontiguousarray(bk[sl].reshape(DL // P, P).T),
                    "bv": np.ascontiguousarray(
                        np.broadcast_to(bv[sl], (P, DL))
                    ).astype(bf),
                    "ones": np.ones((1, D), np.float32),
                }
            )
    return in_maps


def kernel(x, Wq, bq, Wk, bk, Wv, bv, Wp, bp):
    x = np.asarray(x, np.float32)
    Wq, Wk, Wv, Wp = (np.asarray(a, np.float32) for a in (Wq, Wk, Wv, Wp))
    bq, bk, bv, bp = (np.asarray(a, np.float32) for a in (bq, bk, bv, bp))

    nc = build_nc()
    in_maps = _prep_inputs(x, Wq, bq, Wk, bk, Wv, bv, Wp)
    res = run_bass_kernel_spmd(nc, in_maps, core_ids=list(range(8)))

    out = np.empty((B, T, C), np.float32)
    for b in range(B):
        acc = res.results[2 * b]["outT"] + res.results[2 * b + 1]["outT"]
        out[b] = acc.T + bp
    return out



# revision 3
# speedup vs baseline: 3.5930x; 3.5930x over previous
"""v3: interleaved projections+attention, bf16 weights/x, causal shrink,
pool-engine denominator broadcast, software-pipelined PV.

Structure per core (core = 2*b + head-group g, 8 heads x 64 dims):
  upfront: load wk,x -> K(0); wq -> Q(0); wv -> V(0); wp
  for tg in 0..3:
      B(tg): attention over t-group tg, with K/Q/V projection groups of
             t-group tg+1 sprinkled between head-pairs (keeps PE fed while
             ACT chews exps; ACT of B overlaps proj matmuls).
      out-proj(tg)
Score matmuls f32r (kt/qt f32r); everything else bf16 except PSUM paths.
Causal shrink: s-tile si only reaches t >= 128*si; score/exp start at
min(toff, 256) (f32r matmul needs free>=256 for full rate), pv at toff.
Denominator: reciprocal of the ones-column row of psy, broadcast across
64 partitions on the (idle) Pool engine via partition_broadcast.
"""

import sys

if "/opt/trn_rl_repo" not in sys.path:
    sys.path.insert(0, "/opt/trn_rl_repo")

import ml_dtypes
import numpy as np

import concourse.bacc as bacc
import concourse.mybir as mybir
from concourse.tile import TileContext
from concourse.bass_utils import run_bass_kernel_spmd

B, T, C = 4, 2048, 1024
H_LOC = 8
D = 64
DL = H_LOC * D
P = 128
NF = 512
N_TG = T // NF
N_CS = C // P
SCALE = 1.0 / 8.0

F32 = mybir.dt.float32
F32R = mybir.dt.float32r
BF16 = mybir.dt.bfloat16
EXP = mybir.ActivationFunctionType.Exp


def build_nc(pp_bufs=3, pss_bufs=3, ex_bufs=6, psy_bufs=2,
             alloc_mode="stack", repeat=1, interleave=True, sprinkle=3,
             fuse_exp=False, mask_pool=False, bcast_pool=True, tick=False):
    nc = bacc.Bacc("TRN2", target_bir_lowering=False, debug=False, num_devices=8)

    tick_t = (
        nc.dram_tensor("tick", [1, 1], F32, kind="ExternalInput")
        if tick else None
    )
    xT = nc.dram_tensor("xT", [C, T], BF16, kind="ExternalInput")
    wq = nc.dram_tensor("wq", [C, DL], BF16, kind="ExternalInput")
    wk = nc.dram_tensor("wk", [C, DL], BF16, kind="ExternalInput")
    wv = nc.dram_tensor("wv", [C, DL], BF16, kind="ExternalInput")
    wp = nc.dram_tensor("wp", [DL, C], BF16, kind="ExternalInput")
    bq = nc.dram_tensor("bq", [P, DL // P], F32, kind="ExternalInput")
    bk = nc.dram_tensor("bk", [P, DL // P], F32, kind="ExternalInput")
    bv = nc.dram_tensor("bv", [P, DL], BF16, kind="ExternalInput")
    ones_in = nc.dram_tensor("ones", [1, D], F32R, kind="ExternalInput")
    outT = nc.dram_tensor("outT", [C, T], F32, kind="ExternalOutput")

    with TileContext(nc, pool_alloc_mode=alloc_mode) as tc:
        with (
            tc.tile_pool(name="persist", bufs=1) as persist,
            tc.tile_pool(name="wpool", bufs=1) as wpool,
            tc.tile_pool(name="attp", bufs=4) as attp,
            tc.tile_pool(name="ocpp", bufs=2) as ocpp,
            tc.tile_pool(name="att1", bufs=1) as att1,
            tc.tile_pool(name="att2", bufs=2) as att2,
            tc.tile_pool(name="xpool", bufs=1) as xpool,
            tc.tile_pool(name="psum", bufs=2, space="PSUM") as psum,
        ):
            def emit(rep):
                kt_g = [persist.tile([P, DL // P, NF], F32R, tag=f"kt{g}",
                                     name=f"kt{g}_{rep}")
                        for g in range(N_TG)]
                qt_g = [persist.tile([P, DL // P, NF], F32R, tag=f"qt{g}",
                                     name=f"qt{g}_{rep}")
                        for g in range(N_TG)]
                va_g = [persist.tile([P, 4, H_LOC, D + 1], BF16, tag=f"va{g}",
                                     name=f"va{g}_{rep}")
                        for g in range(N_TG)]
                bq_c = persist.tile([P, DL // P], F32, tag="bq")
                bk_c = persist.tile([P, DL // P], F32, tag="bk")
                bv_b = persist.tile([P, DL], BF16, tag="bv")
                ones = persist.tile([P, D], F32R, tag="ones")
                dmask = persist.tile([P, 2 * P], BF16, tag="dmask")

                nc.sync.dma_start(out=bq_c[:], in_=bq[:])
                nc.sync.dma_start(out=bk_c[:], in_=bk[:])
                nc.sync.dma_start(out=bv_b[:], in_=bv[:])
                nc.sync.dma_start(out=ones[D : D + 1, :], in_=ones_in[:])
                # dmask[p, j] = 1 if j - 128 >= p else 0  (diag window at 128)
                nc.vector.memset(dmask[:], 1.0)
                nc.gpsimd.affine_select(
                    out=dmask[:],
                    in_=dmask[:],
                    compare_op=mybir.AluOpType.is_ge,
                    fill=0.0,
                    base=-P,
                    channel_multiplier=-1,
                    pattern=[[1, 2 * P]],
                )
                for g in range(N_TG):
                    nc.vector.memset(va_g[g][:, :, :, D : D + 1], 1.0)
                if bcast_pool and rep == 0:
                    # partition_broadcast lives in the gpsimd `attn` library;
                    # load it after the (native) affine_select mask init
                    from concourse import library_config

                    nc.gpsimd.load_library(library_config.attn)

                # -------- weights + x --------
                wk_sb = wpool.tile([P, N_CS, DL], BF16, tag="wk")
                nc.sync.dma_start(
                    out=wk_sb[:], in_=wk.ap().rearrange("(s p) d -> p s d", p=P)
                )
                xt_c = [xpool.tile([P, T], BF16, tag=f"x{cs}", name=f"x{cs}_{rep}")
                        for cs in range(N_CS)]
                for cs in range(N_CS):
                    nc.sync.dma_start(
                        out=xt_c[cs][:], in_=xT.ap()[cs * P : (cs + 1) * P, :]
                    )
                wq_sb = wpool.tile([P, N_CS, DL], BF16, tag="wq")
                nc.sync.dma_start(
                    out=wq_sb[:], in_=wq.ap().rearrange("(s p) d -> p s d", p=P)
                )
                wv_sb = wpool.tile([P, N_CS, DL], BF16, tag="wv")
                nc.sync.dma_start(
                    out=wv_sb[:], in_=wv.ap().rearrange("(s p) d -> p s d", p=P)
                )
                wp_sb = wpool.tile([P, DL // P, C], BF16, tag="wp")
                nc.sync.dma_start(
                    out=wp_sb[:], in_=wp.ap().rearrange("(s p) c -> p s c", p=P)
                )

                def k_group(g, dt_i):
                    ps = psum.tile([P, NF], F32, tag="pp", bufs=pp_bufs)
                    for cs in range(N_CS):
                        nc.tensor.matmul(
                            ps[:],
                            wk_sb[:, cs, dt_i * P : (dt_i + 1) * P],
                            xt_c[cs][:, g * NF : (g + 1) * NF],
                            start=(cs == 0),
                            stop=(cs == N_CS - 1),
                        )
                    nc.vector.tensor_scalar_add(
                        kt_g[g][:, dt_i, :], ps[:], bk_c[:, dt_i : dt_i + 1]
                    )

                def q_group(g, dt_i):
                    ps = psum.tile([P, NF], F32, tag="pp", bufs=pp_bufs)
                    for cs in range(N_CS):
                        nc.tensor.matmul(
                            ps[:],
                            wq_sb[:, cs, dt_i * P : (dt_i + 1) * P],
                            xt_c[cs][:, g * NF : (g + 1) * NF],
                            start=(cs == 0),
                            stop=(cs == N_CS - 1),
                        )
                    nc.vector.tensor_scalar_add(
                        qt_g[g][:, dt_i, :], ps[:], bq_c[:, dt_i : dt_i + 1]
                    )

                def v_group(st):
                    ps = psum.tile([P, NF], F32, tag="pp", bufs=pp_bufs)
                    for cs in range(N_CS):
                        nc.tensor.matmul(
                            ps[:],
                            xt_c[cs][:, st * P : (st + 1) * P],
                            wv_sb[:, cs, :],
                            start=(cs == 0),
                            stop=(cs == N_CS - 1),
                        )
                    nc.vector.tensor_add(
                        va_g[st // 4][:, st % 4, :, 0:D],
                        ps[:].rearrange("p (h d) -> p h d", d=D),
                        bv_b[:].rearrange("p (h d) -> p h d", d=D),
                    )

                def a_groups(g):
                    for dt_i in range(DL // P):
                        yield lambda dt_i=dt_i: k_group(g, dt_i)
                    for dt_i in range(DL // P):
                        yield lambda dt_i=dt_i: q_group(g, dt_i)
                    for j in range(4):
                        yield lambda j=j: v_group(4 * g + j)

                ytn_g = {}

                def op_group(tg, ct):
                    ytn = ytn_g[tg]
                    pso = psum.tile([P, NF], F32, tag="pp", bufs=pp_bufs)
                    for js in range(DL // P):
                        nc.tensor.matmul(
                            pso[:],
                            wp_sb[:, js, ct * P : (ct + 1) * P],
                            ytn[:, js, :],
                            start=(js == 0),
                            stop=(js == DL // P - 1),
                        )
                    ocp = ocpp.tile([P, NF], F32, tag="ocp")
                    nc.vector.tensor_copy(ocp[:], pso[:])
                    nc.sync.dma_start(
                        out=outT.ap()[
                            ct * P : (ct + 1) * P, tg * NF : (tg + 1) * NF
                        ],
                        in_=ocp[:],
                    )

                def op_groups(tg):
                    for ct in range(C // P):
                        yield lambda ct=ct: op_group(tg, ct)

                def emit_att(tg, feeder):
                    n_s = 4 * (tg + 1)
                    qt = qt_g[tg]
                    ytn = att2.tile([P, DL // P, NF], BF16, tag=f"ytn{tg}",
                                    name=f"ytn{tg}_{rep}", bufs=1)
                    ytn_g[tg] = ytn
                    n_feed = len(feeder)
                    fed = 0
                    for hp in range(H_LOC // 2):
                        pair = (2 * hp, 2 * hp + 1)
                        psy = {
                            h: psum.tile([D + 1, NF], F32, tag="psy",
                                         name=f"psy{h}_t{tg}_{rep}",
                                         bufs=psy_bufs)
                            for h in pair
                        }

                        def flush_pv(si, exs, toff):
                            for h in pair:
                                nc.tensor.matmul(
                                    psy[h][:, toff:] if toff else psy[h],
                                    va_g[si // 4][:, si % 4, h, :],
                                    exs[h][:, toff:],
                                    start=(si == 0),
                                    stop=(si == n_s - 1),
                                )

                        mask_mul = (nc.gpsimd.tensor_mul if mask_pool
                                    else nc.vector.tensor_mul)
                        pend = None  # delayed pv args: (si, exs, toff)
                        for si in range(n_s):
                            toff = max(0, (si - 4 * tg) * P)
                            ts = min(toff, NF - 256)
                            if fuse_exp:
                                # both heads' scores in one 2-bank psum tile;
                                # single exp over [128, 2*(NF-ts)]
                                ps2 = psum.tile([P, 2, NF], F32, tag="pss",
                                                name="ps2", bufs=pss_bufs // 2)
                                pss = {h: ps2[:, i, :] for i, h in enumerate(pair)}
                            else:
                                pss = {}
                                for h in pair:
                                    pss[h] = psum.tile([P, NF], F32, tag="pss",
                                                       name="pss", bufs=pss_bufs)
                            for h in pair:
                                rlo = D * (h % 2)
                                hs = h // 2
                                nc.tensor.matmul(
                                    pss[h][:, ts:],
                                    kt_g[si // 4][
                                        rlo : rlo + D, hs,
                                        (si % 4) * P : (si % 4 + 1) * P
                                    ],
                                    qt[rlo : rlo + D, hs, ts:],
                                    start=True,
                                    stop=True,
                                )
                            exs = {}
                            if fuse_exp:
                                ex2 = attp.tile([P, 2, NF], BF16, tag="ex",
                                                bufs=ex_bufs)
                                nc.scalar.activation(
                                    ex2[:, :, ts:], ps2[:, :, ts:], EXP,
                                    scale=SCALE
                                )
                                for i, h in enumerate(pair):
                                    if si >= 4 * tg:
                                        mask_mul(
                                            ex2[:, i, ts : toff + P],
                                            ex2[:, i, ts : toff + P],
                                            dmask[:, P + ts - toff : 2 * P],
                                        )
                                    exs[h] = ex2[:, i, :]
                            else:
                                for h in pair:
                                    ex = attp.tile([P, NF], BF16, tag="ex",
                                                   bufs=ex_bufs)
                                    nc.scalar.activation(
                                        ex[:, ts:], pss[h][:, ts:], EXP,
                                        scale=SCALE
                                    )
                                    if si >= 4 * tg:  # diagonal: zero s > t
                                        mask_mul(
                                            ex[:, ts : toff + P],
                                            ex[:, ts : toff + P],
                                            dmask[:, P + ts - toff : 2 * P],
                                        )
                                    exs[h] = ex
                            if pend is not None:
                                flush_pv(*pend)
                            pend = (si, exs, toff)
                        flush_pv(*pend)

                        def _norm(h):
                            hs = h // 2
                            rec = att1.tile([D, NF], F32, tag="rec")
                            if bcast_pool:
                                rec1 = att1.tile([1, NF], F32, tag="dt")
                                nc.vector.reciprocal(
                                    rec1[:], psy[h][D : D + 1, :]
                                )
                                nc.gpsimd.partition_broadcast(
                                    rec[:], rec1[0:1, :], channels=D
                                )
                            else:
                                den = att1.tile([D + 1, NF], F32R, tag="dt")
                                nc.vector.tensor_copy(
                                    den[D : D + 1, :], psy[h][D : D + 1, :]
                                )
                                pbc = psum.tile(
                                    [D, NF], F32, name="pbc",
                                    tag="pp" if fuse_exp else "pss",
                                    bufs=pp_bufs if fuse_exp else pss_bufs,
                                )
                                nc.tensor.matmul(
                                    pbc[:],
                                    ones[D : D + 1, :],
                                    den[D : D + 1, :],
                                    start=True,
                                    stop=True,
                                )
                                nc.vector.reciprocal(rec[:], pbc[:])
                            if h % 2 == 0:
                                nc.vector.tensor_mul(
                                    ytn[0:D, hs, :], psy[h][0:D, :], rec[:]
                                )
                            else:
                                tmp = att1.tile([D, NF], BF16, tag="tm")
                                nc.vector.tensor_mul(
                                    tmp[:], psy[h][0:D, :], rec[:]
                                )
                                nc.sync.dma_start(
                                    out=ytn[D:P, hs, :], in_=tmp[:]
                                )

                        for h in pair:
                            _norm(h)
                        # drain an even share of the feeder after each head-pair
                        want = (n_feed * (hp + 1) + 3) // 4
                        while fed < want:
                            feeder[fed]()
                            fed += 1

                # -------- schedule --------
                for dt_i in range(DL // P):
                    k_group(0, dt_i)
                for dt_i in range(DL // P):
                    q_group(0, dt_i)
                for j in range(4):
                    v_group(j)
                for tg in range(N_TG):
                    if tg + 1 < N_TG:
                        feeder = list(a_groups(tg + 1))
                    else:
                        # B3 has no next projections; feed it the deferred
                        # out-projections of t-groups 0..2
                        feeder = [
                            th for t in range(N_TG - 1) for th in op_groups(t)
                        ]
                    if not interleave:
                        for th in feeder:
                            th()
                        feeder = []
                    emit_att(tg, feeder)
                for th in op_groups(N_TG - 1):
                    th()

            if tick_t is not None:
                tick_sb = persist.tile([1, 1], F32, tag="tick")
                nc.sync.dma_start(out=tick_sb[:], in_=tick_t[:])
            emit(0)
            for rep in range(1, repeat):
                emit(rep)

    nc.compile()
    return nc


def _prep_inputs(x, Wq, bq, Wk, bk, Wv, bv, Wp):
    bf = ml_dtypes.bfloat16
    in_maps = []
    for b in range(B):
        xt = np.ascontiguousarray(x[b].T).astype(bf)
        for g in range(2):
            sl = slice(g * DL, (g + 1) * DL)
            in_maps.append(
                {
                    "xT": xt,
                    "wq": np.ascontiguousarray(Wq[:, sl]).astype(bf),
                    "wk": np.ascontiguousarray(Wk[:, sl]).astype(bf),
                    "wv": np.ascontiguousarray(Wv[:, sl]).astype(bf),
                    "wp": np.ascontiguousarray(Wp[sl, :]).astype(bf),
                    "bq": np.ascontiguousarray(bq[sl].reshape(DL // P, P).T),
                    "bk": np.ascontiguousarray(bk[sl].reshape(DL // P, P).T),
                    "bv": np.ascontiguousarray(
                        np.broadcast_to(bv[sl], (P, DL))
                    ).astype(bf),
                    "ones": np.ones((1, D), np.float32),
                }
            )
    return in_maps


def kernel(x, Wq, bq, Wk, bk, Wv, bv, Wp, bp):
    x = np.asarray(x, np.float32)
    Wq, Wk, Wv, Wp = (np.asarray(a, np.float32) for a in (Wq, Wk, Wv, Wp))
    bq, bk, bv, bp = (np.asarray(a, np.float32) for a in (bq, bk, bv, bp))

    nc = build_nc()
    in_maps = _prep_inputs(x, Wq, bq, Wk, bk, Wv, bv, Wp)
    res = run_bass_kernel_spmd(nc, in_maps, core_ids=list(range(8)))

    out = np.empty((B, T, C), np.float32)
    for b in range(B):
        acc = res.results[2 * b]["outT"] + res.results[2 * b + 1]["outT"]
        out[b] = acc.T + bp
    return out

